# revision 8
# baseline (speedup 1.0000x reference)
"""Trainium2 Bass kernel for nn_Conditional_Diffusion_Model (segment_reduce).

Strategy
--------
The reference unconditionally zeroes ``error_pro`` and the ``t == 0`` branch is
dead for real inputs (t is drawn from randint(1, T+1)/T), so the output depends
only on ``error_mol``:

    error_mol[s] = sum_{i in seg s} || p_i - q_s ||^2
    p_i  = raw_i + sigma_s * (raw_i @ W)          raw_i = [x_noise_i | eps_h_i]  (67)
    q_s  = jmcat_s + sigma_s * (jm_s @ W[0:3]) + A_s      (segment-level, 67)

which expands to segment-level quantities only:

    sum ||p||^2 = tr(S_s) + 2 sigma <S_s, W> + sigma^2 <S_s, W W^T>
    sum p       = Sraw_s + sigma * (Sraw_s @ W)

with S_s = sum raw_i^T raw_i (67x67 second moment) and Sraw_s = sum raw_i.
A_s needs the segment mean of xh = [mol_x | mol_h] and jm_s needs the joint
(mol+pro) segment mean of x_noise.

So the device only computes, per segment: S_s, Sraw_s, Sxh_s (one fused PE
matmul per 128-node tile, PSUM-accumulated per segment) and the pro-side
x_noise segment sums.  The host packs nodes into per-segment slabs padded to a
multiple of 128 rows (so each 128-row tile is segment-pure and the SPMD
program is identical on all 8 cores: 128 segments per core) and finishes with
trivial [1024]-vector math.
"""

import os
import sys

import numpy as np

_REPO = "/opt/trn_rl_repo"
if _REPO not in sys.path:
    sys.path.insert(0, _REPO)

B = 1024
XD = 3
NA = 64
T_NORM = 1000.0
NV1 = 4.0
EPS = 1e-10
N_CORES = 8
SEGS_PER_CORE = B // N_CORES  # 128
F = XD + NA  # 67 features in raw / xh
# packed mol row: [x_noise(3) | eps_h(64) | ones(1) | mol_x(3) | mol_h(64)]
MCOLS = F + 1 + F  # 135

TRACE = False  # test harness may flip this to collect an NTFF profile
LAST_EXEC_NS = None

_NC_CACHE = {}


def _build_nc(L, Lp):
    """Build (once per (L, Lp)) the SPMD Bass/Tile program for one core.

    L  : padded rows per mol segment (multiple of 128)
    Lp : padded rows per pro segment (multiple of 128)
    """
    import concourse.bacc as bacc
    import concourse.bass as bass
    import concourse.mybir as mybir
    import concourse.tile as tile

    f32 = mybir.dt.float32
    nc = bacc.Bacc("TRN2", target_bir_lowering=False, debug=False)

    p_mol = nc.dram_tensor("p_mol", [SEGS_PER_CORE * L, MCOLS], f32,
                           kind="ExternalInput")
    p_pro = nc.dram_tensor("p_pro", [SEGS_PER_CORE * Lp, 4], f32,
                           kind="ExternalInput")

    ts = L // 128          # tiles per mol segment
    gp = Lp // 128         # row-groups per pro segment
    spb = 512 // (4 * gp)  # pro segments per PSUM bank row
    n_pro_banks = -(-SEGS_PER_CORE // spb)
    PRO_SEGS_PER_DMA = 4

    out1 = nc.dram_tensor("out1", [SEGS_PER_CORE, F, F + 1], f32,
                          kind="ExternalOutput")
    out2 = nc.dram_tensor("out2", [SEGS_PER_CORE, MCOLS], f32,
                          kind="ExternalOutput")
    out3 = nc.dram_tensor("out3", [n_pro_banks, 512], f32,
                          kind="ExternalOutput")

    mol_ap = p_mol.ap()
    pro_ap = p_pro.ap()
    out1_ap = out1.ap()
    out2_ap = out2.ap()
    out3_ap = out3.ap()

    with tile.TileContext(nc) as tc:
        with (
            tc.tile_pool(name="mol", bufs=3) as mol_pool,
            tc.tile_pool(name="pro", bufs=2) as pro_pool,
            tc.tile_pool(name="res", bufs=4) as res_pool,
            tc.tile_pool(name="const", bufs=1) as const_pool,
            tc.tile_pool(name="psum", bufs=4, space=bass.MemorySpace.PSUM) as psum_pool,
            tc.tile_pool(name="psum_pro", bufs=2, space=bass.MemorySpace.PSUM) as psum_pro_pool,
        ):
            ones = const_pool.tile([128, 1], f32)
            nc.vector.memset(ones[:], 1.0)

            # ---- mol pass: per segment, accumulate [raw|ones]^T @ [raw|ones|xh]
            for s in range(SEGS_PER_CORE):
                tile_in = mol_pool.tile([128, ts, MCOLS], f32)
                src = mol_ap[s * L:(s + 1) * L, :].rearrange(
                    "(t p) f -> p t f", p=128)
                nc.sync.dma_start(out=tile_in[:], in_=src)
                ps = psum_pool.tile([F + 1, MCOLS], f32)
                for j in range(ts):
                    nc.tensor.matmul(
                        ps[:, :],
                        tile_in[:, j, 0:F + 1],
                        tile_in[:, j, :],
                        start=(j == 0),
                        stop=(j == ts - 1),
                    )
                # rows 0:67 cols 0:68 = [S | Sraw];  row 67 = [Sraw | n | Sxh]
                res = res_pool.tile([F + 1, MCOLS], f32)
                nc.vector.tensor_copy(res[:, :], ps[:, :])
                nc.sync.dma_start(out=out1_ap[s], in_=res[0:F, 0:F + 1])
                nc.sync.dma_start(out=out2_ap[s:s + 1, :], in_=res[F:F + 1, :])

            # ---- pro pass: ones^T @ [x|y|z|0] per segment (joint-mean sums)
            pro_ps = None
            for s in range(SEGS_PER_CORE):
                if s % PRO_SEGS_PER_DMA == 0:
                    n_seg = min(PRO_SEGS_PER_DMA, SEGS_PER_CORE - s)
                    pro_tile = pro_pool.tile([128, PRO_SEGS_PER_DMA * gp, 4], f32)
                    src = pro_ap[s * Lp:(s + n_seg) * Lp, :].rearrange(
                        "(t p) f -> p t f", p=128)
                    nc.sync.dma_start(out=pro_tile[:, 0:n_seg * gp, :], in_=src)
                bank = s // spb
                slot = s % spb
                if slot == 0:
                    pro_ps = psum_pro_pool.tile([1, 512], f32)
                j0 = (s % PRO_SEGS_PER_DMA) * gp
                nc.tensor.matmul(
                    pro_ps[:, slot * 4 * gp:(slot + 1) * 4 * gp],
                    ones[:, :],
                    pro_tile[:, j0:j0 + gp, :],
                    start=True,
                    stop=True,
                )
                if slot == spb - 1 or s == SEGS_PER_CORE - 1:
                    pres = res_pool.tile([1, 512], f32, tag="pro_res")
                    nc.vector.tensor_copy(pres[:, :], pro_ps[:, :])
                    nc.sync.dma_start(out=out3_ap[bank:bank + 1, :],
                                      in_=pres[:, :])

    nc.compile()
    return nc


def _segment_starts(idx, n_rows):
    return np.searchsorted(idx, np.arange(B + 1), side="left").astype(np.int64)


def _pad_len(max_cnt, floor_val):
    L = max(int(floor_val), int(-(-max_cnt // 128) * 128))
    return max(L, 128)


def kernel(**inputs):
    global LAST_EXEC_NS
    from concourse.bass_utils import run_bass_kernel_spmd

    mol_x = np.asarray(inputs["mol_x"], dtype=np.float32)
    mol_h = np.asarray(inputs["mol_h"], dtype=np.float32)
    Wm = np.asarray(inputs["Wm"], dtype=np.float32)
    bm = np.asarray(inputs["bm"], dtype=np.float32)
    t = np.asarray(inputs["t"], dtype=np.float32)
    x_noise = np.asarray(inputs["x_noise"], dtype=np.float32)
    eps_h_mol = np.asarray(inputs["eps_h_mol"], dtype=np.float32)
    mol_idx = np.asarray(inputs["mol_idx"]).astype(np.int64)
    pro_idx = np.asarray(inputs["pro_idx"]).astype(np.int64)

    Nm = mol_x.shape[0]
    Np = pro_idx.shape[0]
    xn_mol = x_noise[:Nm]
    xn_pro = x_noise[Nm:]

    m_start = _segment_starts(mol_idx, Nm)
    p_start = _segment_starts(pro_idx, Np)
    cnt_m = np.diff(m_start)
    cnt_p = np.diff(p_start)

    L = _pad_len(cnt_m.max(), 512)
    Lp = _pad_len(cnt_p.max(), 1024)

    # ---- host pack: per-segment slabs, zero padded ----
    P = np.zeros((B * L, MCOLS), dtype=np.float32)
    dest = np.arange(Nm, dtype=np.int64) - m_start[mol_idx] + mol_idx * L
    P[dest, 0:XD] = xn_mol
    P[dest, XD:F] = eps_h_mol
    P[dest, F] = 1.0
    P[dest, F + 1:F + 1 + XD] = mol_x
    P[dest, F + 1 + XD:MCOLS] = mol_h

    Ppro = np.zeros((B * Lp, 4), dtype=np.float32)
    dest_p = np.arange(Np, dtype=np.int64) - p_start[pro_idx] + pro_idx * Lp
    Ppro[dest_p, 0:XD] = xn_pro

    key = (L, Lp)
    if key not in _NC_CACHE:
        _NC_CACHE[key] = _build_nc(L, Lp)
    nc = _NC_CACHE[key]

    rows_m = SEGS_PER_CORE * L
    rows_p = SEGS_PER_CORE * Lp
    in_maps = [
        {
            "p_mol": P[c * rows_m:(c + 1) * rows_m],
            "p_pro": Ppro[c * rows_p:(c + 1) * rows_p],
        }
        for c in range(N_CORES)
    ]

    res = run_bass_kernel_spmd(nc, in_maps, core_ids=list(range(N_CORES)),
                               trace=TRACE)
    LAST_EXEC_NS = res.exec_time_ns

    gp = Lp // 128
    spb = 512 // (4 * gp)
    S = np.empty((B, F, F), dtype=np.float64)
    Sraw = np.empty((B, F), dtype=np.float64)
    Sxh = np.empty((B, F), dtype=np.float64)
    Sxp = np.empty((B, XD), dtype=np.float64)
    for c in range(N_CORES):
        o1 = np.asarray(res.results[c]["out1"], dtype=np.float64)
        o2 = np.asarray(res.results[c]["out2"], dtype=np.float64)
        o3 = np.asarray(res.results[c]["out3"], dtype=np.float64)
        sl = slice(c * SEGS_PER_CORE, (c + 1) * SEGS_PER_CORE)
        S[sl] = o1[:, :, 0:F]
        Sraw[sl] = o2[:, 0:F]
        Sxh[sl] = o2[:, F + 1:MCOLS]
        for s in range(SEGS_PER_CORE):
            bank = s // spb
            col = (s % spb) * 4 * gp
            blk = o3[bank, col:col + 4 * gp].reshape(gp, 4)
            Sxp[c * SEGS_PER_CORE + s] = blk[:, 0:XD].sum(axis=0)

    # ---- segment-level finale (host, [1024]-sized) ----
    W = Wm[:F, :].astype(np.float64)
    wrow = Wm[F, :].astype(np.float64)
    bmv = bm.astype(np.float64)
    t_s = t[:, 0].astype(np.float64)
    n_m = cnt_m.astype(np.float64)
    n_p = cnt_p.astype(np.float64)

    alpha = 1.0 - (t_s / T_NORM) ** 2
    sigma = np.sqrt(1.0 - alpha ** 2)

    m_mean = np.empty((B, F))
    m_mean[:, 0:XD] = Sxh[:, 0:XD] / n_m[:, None]
    m_mean[:, XD:F] = Sxh[:, XD:F] / (NV1 * n_m[:, None])
    jm = (Sraw[:, 0:XD] + Sxp) / (n_m + n_p)[:, None]

    A = alpha[:, None] * (m_mean @ W) + t_s[:, None] * wrow[None, :] + bmv[None, :]
    q = np.concatenate([jm, np.zeros((B, F - XD))], axis=1) \
        + sigma[:, None] * (jm @ W[0:XD, :]) + A
    Sp = Sraw + sigma[:, None] * (Sraw @ W)

    trS = np.trace(S, axis1=1, axis2=2)
    bS = np.einsum("sij,ij->s", S, W)
    cS = np.einsum("sij,ij->s", S, W @ W.T)
    sum_p2 = trS + 2.0 * sigma * bS + sigma ** 2 * cS

    err = sum_p2 - 2.0 * np.einsum("sf,sf->s", q, Sp) + n_m * np.einsum("sf,sf->s", q, q)

    t0 = (t_s == 0.0).astype(np.float64)
    tn0 = 1.0 - t0
    error_mol = err * tn0 / ((Nm + NA) * n_m)
    loss = 0.5 * error_mol

    if np.any(t0 > 0):
        loss = loss + _t0_terms(mol_x, mol_h, xn_mol, eps_h_mol, mol_idx,
                                m_start, cnt_m, Wm, bm, t_s, jm, alpha, sigma,
                                t0, Nm)

    return np.float32(loss.mean())


def _t0_terms(mol_x, mol_h, xn_mol, eps_h_mol, mol_idx, m_start, cnt_m,
              Wm, bm, t_s, jm, alpha, sigma, t0, Nm):
    """Faithful numpy port of the t==0 branch (never taken for real inputs).

    Returns the [B] vector loss_0 = loss_x_mol_t0 + loss_h_t0 exactly as the
    reference computes it (zero wherever t != 0).
    """
    try:
        from scipy.special import erf
    except Exception:
        import math
        erf = np.vectorize(math.erf)

    W = Wm.astype(np.float64)
    n_m = cnt_m.astype(np.float64)
    mol_h_sc = mol_h.astype(np.float64) / NV1
    xh = np.concatenate([mol_x.astype(np.float64), mol_h_sc], axis=1)
    m_mean = np.add.reduceat(xh, m_start[:-1], axis=0) / n_m[:, None]

    eps_x = xn_mol.astype(np.float64) - jm[mol_idx]
    epsilon = np.concatenate([eps_x, eps_h_mol.astype(np.float64)], axis=1)
    z_t = alpha[mol_idx, None] * m_mean[mol_idx] - sigma[mol_idx, None] * epsilon
    eps_hat = (np.concatenate([z_t, t_s[mol_idx, None]], axis=1) @ W
               + bm.astype(np.float64)[None, :])

    seg = lambda v: np.add.reduceat(v, m_start[:-1])
    loss_x_t0 = -0.5 * seg(((epsilon[:, :XD] - eps_hat[:, :XD]) ** 2).sum(axis=1))

    sigma_0_un = sigma * NV1
    mol_h_hat = z_t[:, XD:] * NV1
    c = mol_h_hat - 1.0
    s = sigma_0_un[mol_idx][:, None]
    sqrt2 = np.sqrt(2.0)
    logp_un = np.log(0.5 * (1.0 + erf((c + 0.5) / s) / sqrt2)
                     - 0.5 * (1.0 + erf((c - 0.5) / s) / sqrt2) + EPS)
    mx = logp_un.max(axis=1, keepdims=True)
    lse = mx + np.log(np.exp(logp_un - mx).sum(axis=1, keepdims=True))
    logp = logp_un - lse
    loss_h_t0 = seg((logp * mol_h_sc).sum(axis=1))

    loss_x_t0 = -loss_x_t0 * t0 / Nm * n_m
    loss_h_t0 = -loss_h_t0 * t0
    return loss_x_t0 + loss_h_t0


if __name__ == "__main__":
    pass


# revision 11
# speedup vs baseline: 2.6553x; 2.6553x over previous
"""Trainium2 Bass kernel for nn_Conditional_Diffusion_Model (segment_reduce).

Strategy
--------
The reference unconditionally zeroes ``error_pro`` and the ``t == 0`` branch is
dead for real inputs (t is drawn from randint(1, T+1)/T), so the output depends
only on ``error_mol``:

    error_mol[s] = sum_{i in seg s} || p_i - q_s ||^2
    p_i  = raw_i + sigma_s * (raw_i @ W)          raw_i = [x_noise_i | eps_h_i]  (67)
    q_s  = jmcat_s + sigma_s * (jm_s @ W[0:3]) + A_s      (segment-level, 67)

which expands to segment-level quantities only:

    sum ||p||^2 = tr(S_s) + 2 sigma <S_s, W> + sigma^2 <S_s, W W^T>
    sum p       = Sraw_s + sigma * (Sraw_s @ W)

with S_s = sum raw_i^T raw_i (67x67 second moment) and Sraw_s = sum raw_i.
A_s needs the segment mean of xh = [mol_x | mol_h] and jm_s needs the joint
(mol+pro) segment mean of x_noise.

So the device only computes, per segment: S_s, Sraw_s, Sxh_s (one fused PE
matmul per 128-node tile, PSUM-accumulated per segment) and the pro-side
x_noise segment sums.  The host packs nodes into per-segment slabs padded to a
multiple of 128 rows (so each 128-row tile is segment-pure and the SPMD
program is identical on all 8 cores: 128 segments per core) and finishes with
trivial [1024]-vector math.
"""

import os
import sys

import numpy as np

_REPO = "/opt/trn_rl_repo"
if _REPO not in sys.path:
    sys.path.insert(0, _REPO)

B = 1024
XD = 3
NA = 64
T_NORM = 1000.0
NV1 = 4.0
EPS = 1e-10
N_CORES = 8
SEGS_PER_CORE = B // N_CORES  # 128
F = XD + NA  # 67 features in raw / xh
# packed mol row: [x_noise(3) | eps_h(64) | ones(1) | mol_x(3) | mol_h(64)]
MCOLS = F + 1 + F  # 135

TRACE = False  # test harness may flip this to collect an NTFF profile
LAST_EXEC_NS = None

_NC_CACHE = {}


def _build_nc(L, Lp):
    """Build (once per (L, Lp)) the SPMD Bass/Tile program for one core.

    L  : padded rows per mol segment (multiple of 128)
    Lp : padded rows per pro segment (multiple of 128)
    """
    import concourse.bacc as bacc
    import concourse.bass as bass
    import concourse.mybir as mybir
    import concourse.tile as tile

    f32 = mybir.dt.float32
    nc = bacc.Bacc("TRN2", target_bir_lowering=False, debug=False)

    ts = L // 128          # 128-row tiles per mol segment
    gp = Lp // 128         # 128-row tiles per pro segment
    spb = 512 // (4 * gp)  # pro segments per PSUM bank
    n_pro_banks = -(-SEGS_PER_CORE // spb)
    SG = 8                 # mol segments per input DMA
    PSG = 32               # pro segments per input DMA
    OSG = 32               # segments per output DMA

    # partition-major packed inputs: [128, SEGS*ts, MCOLS] flattened
    p_mol = nc.dram_tensor("p_mol", [128 * SEGS_PER_CORE * ts, MCOLS], f32,
                           kind="ExternalInput")
    p_pro = nc.dram_tensor("p_pro", [128 * SEGS_PER_CORE * gp, 4], f32,
                           kind="ExternalInput")

    out1 = nc.dram_tensor("out1", [F + 1, SEGS_PER_CORE, MCOLS], f32,
                          kind="ExternalOutput")
    out3 = nc.dram_tensor("out3", [1, n_pro_banks * 512], f32,
                          kind="ExternalOutput")

    mol_ap = p_mol.ap().rearrange("(p n) f -> p n f", p=128)
    pro_ap = p_pro.ap().rearrange("(p n) f -> p n f", p=128)
    out1_ap = out1.ap()
    out3_ap = out3.ap()

    with tile.TileContext(nc) as tc:
        with (
            tc.tile_pool(name="mol", bufs=3) as mol_pool,
            tc.tile_pool(name="pro", bufs=2) as pro_pool,
            tc.tile_pool(name="res", bufs=1) as res_pool,
            tc.tile_pool(name="const", bufs=1) as const_pool,
            tc.tile_pool(name="psum", bufs=4, space=bass.MemorySpace.PSUM) as psum_pool,
            tc.tile_pool(name="psum_pro", bufs=2, space=bass.MemorySpace.PSUM) as psum_pro_pool,
        ):
            ones = const_pool.tile([128, 1], f32)
            nc.vector.memset(ones[:], 1.0)

            # persistent result accumulators, flushed with few large DMAs
            res_all = res_pool.tile([F + 1, SEGS_PER_CORE, MCOLS], f32)
            pro_res = res_pool.tile([1, n_pro_banks * 512], f32, tag="pro_res")

            # ---- mol pass: per segment, accumulate [raw|ones]^T @ [raw|ones|xh]
            for s0 in range(0, SEGS_PER_CORE, SG):
                tile_in = mol_pool.tile([128, SG * ts, MCOLS], f32)
                nc.sync.dma_start(
                    out=tile_in[:],
                    in_=mol_ap[:, s0 * ts:(s0 + SG) * ts, :])
                for sl in range(SG):
                    s = s0 + sl
                    ps = psum_pool.tile([F + 1, MCOLS], f32)
                    for j in range(ts):
                        nc.tensor.matmul(
                            ps[:, :],
                            tile_in[:, sl * ts + j, 0:F + 1],
                            tile_in[:, sl * ts + j, :],
                            start=(j == 0),
                            stop=(j == ts - 1),
                        )
                    # [0:67, 0:67]=S, [0:67, 67]=Sraw, row 67=[Sraw|n|Sxh]
                    nc.vector.tensor_copy(res_all[:, s, :], ps[:, :])

            # ---- pro pass: ones^T @ [x|y|z|0] per segment (joint-mean sums)
            pro_ps = None
            for s in range(SEGS_PER_CORE):
                if s % PSG == 0:
                    pro_tile = pro_pool.tile([128, PSG * gp, 4], f32)
                    nc.sync.dma_start(
                        out=pro_tile[:],
                        in_=pro_ap[:, s * gp:(s + PSG) * gp, :])
                bank = s // spb
                slot = s % spb
                if slot == 0:
                    pro_ps = psum_pro_pool.tile([1, 512], f32)
                j0 = (s % PSG) * gp
                nc.tensor.matmul(
                    pro_ps[:, slot * 4 * gp:(slot + 1) * 4 * gp],
                    ones[:, :],
                    pro_tile[:, j0:j0 + gp, :],
                    start=True,
                    stop=True,
                )
                if slot == spb - 1 or s == SEGS_PER_CORE - 1:
                    nc.vector.tensor_copy(
                        pro_res[:, bank * 512:(bank + 1) * 512], pro_ps[:, :])

            # ---- flush results ----
            for s0 in range(0, SEGS_PER_CORE, OSG):
                nc.sync.dma_start(out=out1_ap[:, s0:s0 + OSG, :],
                                  in_=res_all[:, s0:s0 + OSG, :])
            nc.sync.dma_start(out=out3_ap[:], in_=pro_res[:])

    nc.compile()
    return nc


def _segment_starts(idx, n_rows):
    return np.searchsorted(idx, np.arange(B + 1), side="left").astype(np.int64)


def _pad_len(max_cnt, floor_val):
    L = max(int(floor_val), int(-(-max_cnt // 128) * 128))
    return max(L, 128)


def kernel(**inputs):
    global LAST_EXEC_NS
    from concourse.bass_utils import run_bass_kernel_spmd

    mol_x = np.asarray(inputs["mol_x"], dtype=np.float32)
    mol_h = np.asarray(inputs["mol_h"], dtype=np.float32)
    Wm = np.asarray(inputs["Wm"], dtype=np.float32)
    bm = np.asarray(inputs["bm"], dtype=np.float32)
    t = np.asarray(inputs["t"], dtype=np.float32)
    x_noise = np.asarray(inputs["x_noise"], dtype=np.float32)
    eps_h_mol = np.asarray(inputs["eps_h_mol"], dtype=np.float32)
    mol_idx = np.asarray(inputs["mol_idx"]).astype(np.int64)
    pro_idx = np.asarray(inputs["pro_idx"]).astype(np.int64)

    Nm = mol_x.shape[0]
    Np = pro_idx.shape[0]
    xn_mol = x_noise[:Nm]
    xn_pro = x_noise[Nm:]

    m_start = _segment_starts(mol_idx, Nm)
    p_start = _segment_starts(pro_idx, Np)
    cnt_m = np.diff(m_start)
    cnt_p = np.diff(p_start)

    L = _pad_len(cnt_m.max(), 512)
    Lp = _pad_len(cnt_p.max(), 1024)

    # ---- host pack: partition-major per-segment slabs, zero padded ----
    # per-core layout [128 partitions, SEGS*ts tiles, MCOLS]; node with
    # in-segment index k lands at partition k%128, tile s_local*ts + k//128.
    ts = L // 128
    gp = Lp // 128
    k_m = np.arange(Nm, dtype=np.int64) - m_start[mol_idx]
    core_m = mol_idx // SEGS_PER_CORE
    sl_m = mol_idx % SEGS_PER_CORE
    npc_m = 128 * SEGS_PER_CORE * ts  # rows per core
    dest = (core_m * npc_m + (k_m % 128) * (SEGS_PER_CORE * ts)
            + sl_m * ts + k_m // 128)
    P = np.zeros((N_CORES * npc_m, MCOLS), dtype=np.float32)
    P[dest, 0:XD] = xn_mol
    P[dest, XD:F] = eps_h_mol
    P[dest, F] = 1.0
    P[dest, F + 1:F + 1 + XD] = mol_x
    P[dest, F + 1 + XD:MCOLS] = mol_h

    k_p = np.arange(Np, dtype=np.int64) - p_start[pro_idx]
    core_p = pro_idx // SEGS_PER_CORE
    sl_p = pro_idx % SEGS_PER_CORE
    npc_p = 128 * SEGS_PER_CORE * gp
    dest_p = (core_p * npc_p + (k_p % 128) * (SEGS_PER_CORE * gp)
              + sl_p * gp + k_p // 128)
    Ppro = np.zeros((N_CORES * npc_p, 4), dtype=np.float32)
    Ppro[dest_p, 0:XD] = xn_pro

    key = (L, Lp)
    if key not in _NC_CACHE:
        _NC_CACHE[key] = _build_nc(L, Lp)
    nc = _NC_CACHE[key]

    in_maps = [
        {
            "p_mol": P[c * npc_m:(c + 1) * npc_m],
            "p_pro": Ppro[c * npc_p:(c + 1) * npc_p],
        }
        for c in range(N_CORES)
    ]

    res = run_bass_kernel_spmd(nc, in_maps, core_ids=list(range(N_CORES)),
                               trace=TRACE)
    LAST_EXEC_NS = res.exec_time_ns

    spb = 512 // (4 * gp)
    S = np.empty((B, F, F), dtype=np.float64)
    Sraw = np.empty((B, F), dtype=np.float64)
    Sxh = np.empty((B, F), dtype=np.float64)
    Sxp = np.empty((B, XD), dtype=np.float64)
    for c in range(N_CORES):
        # out1: [68, SEGS, 135]; [0:67,s,0:67]=S, row 67 = [Sraw | n | Sxh]
        o1 = np.asarray(res.results[c]["out1"], dtype=np.float64)
        o3 = np.asarray(res.results[c]["out3"], dtype=np.float64)[0]
        sl = slice(c * SEGS_PER_CORE, (c + 1) * SEGS_PER_CORE)
        S[sl] = o1[0:F, :, 0:F].transpose(1, 0, 2)
        Sraw[sl] = o1[F, :, 0:F]
        Sxh[sl] = o1[F, :, F + 1:MCOLS]
        for s in range(SEGS_PER_CORE):
            bank = s // spb
            col = bank * 512 + (s % spb) * 4 * gp
            blk = o3[col:col + 4 * gp].reshape(gp, 4)
            Sxp[c * SEGS_PER_CORE + s] = blk[:, 0:XD].sum(axis=0)

    # ---- segment-level finale (host, [1024]-sized) ----
    W = Wm[:F, :].astype(np.float64)
    wrow = Wm[F, :].astype(np.float64)
    bmv = bm.astype(np.float64)
    t_s = t[:, 0].astype(np.float64)
    n_m = cnt_m.astype(np.float64)
    n_p = cnt_p.astype(np.float64)

    alpha = 1.0 - (t_s / T_NORM) ** 2
    sigma = np.sqrt(1.0 - alpha ** 2)

    m_mean = np.empty((B, F))
    m_mean[:, 0:XD] = Sxh[:, 0:XD] / n_m[:, None]
    m_mean[:, XD:F] = Sxh[:, XD:F] / (NV1 * n_m[:, None])
    jm = (Sraw[:, 0:XD] + Sxp) / (n_m + n_p)[:, None]

    A = alpha[:, None] * (m_mean @ W) + t_s[:, None] * wrow[None, :] + bmv[None, :]
    q = np.concatenate([jm, np.zeros((B, F - XD))], axis=1) \
        + sigma[:, None] * (jm @ W[0:XD, :]) + A
    Sp = Sraw + sigma[:, None] * (Sraw @ W)

    trS = np.trace(S, axis1=1, axis2=2)
    bS = np.einsum("sij,ij->s", S, W)
    cS = np.einsum("sij,ij->s", S, W @ W.T)
    sum_p2 = trS + 2.0 * sigma * bS + sigma ** 2 * cS

    err = sum_p2 - 2.0 * np.einsum("sf,sf->s", q, Sp) + n_m * np.einsum("sf,sf->s", q, q)

    t0 = (t_s == 0.0).astype(np.float64)
    tn0 = 1.0 - t0
    error_mol = err * tn0 / ((Nm + NA) * n_m)
    loss = 0.5 * error_mol

    if np.any(t0 > 0):
        loss = loss + _t0_terms(mol_x, mol_h, xn_mol, eps_h_mol, mol_idx,
                                m_start, cnt_m, Wm, bm, t_s, jm, alpha, sigma,
                                t0, Nm)

    return np.float32(loss.mean())


def _t0_terms(mol_x, mol_h, xn_mol, eps_h_mol, mol_idx, m_start, cnt_m,
              Wm, bm, t_s, jm, alpha, sigma, t0, Nm):
    """Faithful numpy port of the t==0 branch (never taken for real inputs).

    Returns the [B] vector loss_0 = loss_x_mol_t0 + loss_h_t0 exactly as the
    reference computes it (zero wherever t != 0).
    """
    try:
        from scipy.special import erf
    except Exception:
        import math
        erf = np.vectorize(math.erf)

    W = Wm.astype(np.float64)
    n_m = cnt_m.astype(np.float64)
    mol_h_sc = mol_h.astype(np.float64) / NV1
    xh = np.concatenate([mol_x.astype(np.float64), mol_h_sc], axis=1)
    m_mean = np.add.reduceat(xh, m_start[:-1], axis=0) / n_m[:, None]

    eps_x = xn_mol.astype(np.float64) - jm[mol_idx]
    epsilon = np.concatenate([eps_x, eps_h_mol.astype(np.float64)], axis=1)
    z_t = alpha[mol_idx, None] * m_mean[mol_idx] - sigma[mol_idx, None] * epsilon
    eps_hat = (np.concatenate([z_t, t_s[mol_idx, None]], axis=1) @ W
               + bm.astype(np.float64)[None, :])

    seg = lambda v: np.add.reduceat(v, m_start[:-1])
    loss_x_t0 = -0.5 * seg(((epsilon[:, :XD] - eps_hat[:, :XD]) ** 2).sum(axis=1))

    sigma_0_un = sigma * NV1
    mol_h_hat = z_t[:, XD:] * NV1
    c = mol_h_hat - 1.0
    s = sigma_0_un[mol_idx][:, None]
    sqrt2 = np.sqrt(2.0)
    logp_un = np.log(0.5 * (1.0 + erf((c + 0.5) / s) / sqrt2)
                     - 0.5 * (1.0 + erf((c - 0.5) / s) / sqrt2) + EPS)
    mx = logp_un.max(axis=1, keepdims=True)
    lse = mx + np.log(np.exp(logp_un - mx).sum(axis=1, keepdims=True))
    logp = logp_un - lse
    loss_h_t0 = seg((logp * mol_h_sc).sum(axis=1))

    loss_x_t0 = -loss_x_t0 * t0 / Nm * n_m
    loss_h_t0 = -loss_h_t0 * t0
    return loss_x_t0 + loss_h_t0


if __name__ == "__main__":
    pass
